# revision 18
# baseline (speedup 1.0000x reference)
"""ExpertGNN Trainium2 kernel (8 NeuronCores, data-parallel over batch).

Reference computation (B=64, N=4096 nodes on a 64x64 grid, HIDDEN=128):
    h0 = gelu(LN(x_nodes @ W0) * g0 + be0)
    h_{l+1} = gelu(LN((adj @ h_l) @ W_l) * g_l + be_l)   l = 1..3
    out = LN((h3 + h0) @ Wo) * go + beo                  -> [B, 64, 64, 64]

Key structural tricks:
  * adj is a banded block matrix (|i-j| <= 2 tiles of 128 nodes): 154 dense
    128x128 blocks, stored as per-(group, source-tile) strips. Message
    matmuls keep h tiles stationary and produce the channel-major msg^T for
    free, feeding the W matmul with no transposes. PSUM per-element
    has_written gives the sliding-window accumulation with no zero padding.
  * LN mean-centering is folded into the weights on the host (W' = W @ P,
    P = I - 1/C), so the device only needs the variance.
  * Variance: per-tile bn_stats (even/odd partials) combined + bit-trick
    rsqrt (seed + 1 Newton) on the vector engine, batched per layer
    ([128, 32] chain ops). The scalar engine only runs Copy/Gelu -> a single
    activation-table load for the whole kernel.
  * The embed layer's variance is a quadratic form in the 3 input channels:
    computed by the PE from 6 host-precomputed quadratic channels via an
    extra N=1 matmul per tile (no bn_stats for the embed).
  * Elementwise runs at 8-tile width with a stride-0 broadcast view of the
    per-node rstd: one tensor_tensor replaces 8 per-tile tensor_scalars
    (per-instruction overhead ~220ns dominates narrow DVE ops).
"""

import numpy as np
import ml_dtypes

import bass_rust
import concourse.bass as bass
import concourse.mybir as mybir
from concourse.tile import TileContext
from concourse.vector_clock import ScopedClock
from concourse import bass_utils

# ---------------------------------------------------------------- constants
B = 64
N_CORES = 8
B_LOC = B // N_CORES          # 8 batch elements per core
GRID = 64
N = GRID * GRID               # 4096 nodes
NT = 32                       # node tiles of 128
HID = 128
OUT_C = 64
IN_C = 3
AUG_C = 9                     # x channels + quadratic monomials
LN_EPS = 1e-5
GRP = 4                       # node tiles per psum group
NGRP = NT // GRP              # 8 groups per batch-stage
PAIR = 8                      # node tiles per elementwise pair-group
NPAIR = NT // PAIR            # 4
MAGIC = 0x5F3759DF - 0x400000  # rsqrt seed magic, adjusted for vh = v/2 input

F32 = mybir.dt.float32
BF16 = mybir.dt.bfloat16
I32 = mybir.dt.int32
AF = mybir.ActivationFunctionType
ALU = mybir.AluOpType

# strip table: for group g (block-cols 4g..4g+3), source tiles i with the
# contiguous block-col range [j0, j1] they feed.
STRIPS = {}
_slot = 0
ADJ_SLOTS = {}
for _g in range(NGRP):
    lst = []
    for _i in range(max(0, 4 * _g - 2), min(NT, 4 * _g + 6)):
        _j0 = max(4 * _g, _i - 2)
        _j1 = min(4 * _g + 3, _i + 2)
        lst.append((_i, _j0, _j1, _slot))
        for _j in range(_j0, _j1 + 1):
            ADJ_SLOTS[(_i, _j)] = _slot + (_j - _j0)
        _slot += _j1 - _j0 + 1
    STRIPS[_g] = lst
N_BLK = _slot                 # 154


# ------------------------------------------------- walrus drain workaround
def _patched_drain_and_barrier(self, tick_clock, wait_clock):
    """Move tail-drain sem waits onto individual SP nops: this walrus build
    rejects a Drain carrying more than one sync wait."""
    probe = self.nc.sync.nop(nofuse=True)
    wait_clock.add_sem_waits(probe.ins, ScopedClock({None: tick_clock.global_clock}))
    si = probe.ins.sync_info
    if si is not None and len(si.on_wait) > 1:
        waits = list(si.on_wait)
        probe.ins.sync_info = bass_rust.SyncInfo(
            on_wait=waits[:1], on_update=list(si.on_update)
        )
        for w in waits[1:]:
            extra = self.nc.sync.nop(nofuse=True)
            extra.ins.sync_info = bass_rust.SyncInfo(on_wait=[w], on_update=[])
    self.nc.sync.drain()
    self.nc.all_engine_barrier()
    assert self.sems is not None
    popped = self.nc._tile_sem_poison_stack.pop()
    assert popped is self._sem_poison
    self.nc.clear_and_free_semaphores(list(self.sems.allocated().values()))
    self.nc.all_engine_barrier()


TileContext._drain_and_barrier = _patched_drain_and_barrier


def _split_multi_waits(nc, max_waits=1):
    """This walrus build rejects instructions carrying more than one sync
    wait; peel extras onto same-engine NoOps inserted just before."""
    n_split = 0
    for f in nc.m.functions:
        for blk in f.blocks:
            il = blk.instructions
            out = []
            changed = False
            for inst in il:
                si = inst.sync_info
                if si is not None and len(si.on_wait) > max_waits:
                    waits = list(si.on_wait)
                    for k, w in enumerate(waits[: len(waits) - max_waits]):
                        nop = bass_rust.InstNoOp(name=f"{inst.name}-sw{k}")
                        nop.engine = inst.engine
                        nop.sync_info = bass_rust.SyncInfo(on_wait=[w], on_update=[])
                        out.append(nop)
                    inst.sync_info = bass_rust.SyncInfo(
                        on_wait=waits[len(waits) - max_waits :],
                        on_update=list(si.on_update),
                    )
                    changed = True
                    n_split += 1
                out.append(inst)
            if changed:
                blk.instructions = out
    return n_split


# ----------------------------------------------------------- device program
def _build_program():
    nc = bass.Bass(trn_type="TRN2", target_bir_lowering=False, debug=False)

    def din(name, shape, dt):
        return nc.dram_tensor(name, shape, dt, kind="ExternalInput").ap()

    x_d = din("xaug", [B_LOC, AUG_C, NT, 128], BF16)
    adj_d = din("adjS", [128, N_BLK, 128], BF16)
    w0_d = din("w0aug", [AUG_C, HID], BF16)
    wv_d = din("wv", [AUG_C, 1], BF16)
    wl_d = [din(f"w{l}", [HID, HID], BF16) for l in (1, 2, 3)]
    wo_d = din("wo", [HID, OUT_C], BF16)
    g_d = [din(f"g{l}B", [128, PAIR, HID], BF16) for l in range(4)]
    be_d = [din(f"be{l}B", [128, PAIR, HID], BF16) for l in range(4)]
    go_d = din("go", [128, 1], F32)
    beo_d = din("beo", [128, 1], F32)
    idb_d = din("id_bf", [128, 128], BF16)
    out_d = nc.dram_tensor(
        "out", [B_LOC, OUT_C, NT // 2, 2, 128], F32, kind="ExternalOutput"
    ).ap()

    with TileContext(nc) as tc:
        with (
            tc.tile_pool(name="const", bufs=1) as cp,
            tc.tile_pool(name="hbuf", bufs=2) as hp,
            tc.tile_pool(name="xin", bufs=2) as xp,
            tc.tile_pool(name="osb", bufs=2) as op_,
            tc.tile_pool(name="mts", bufs=3) as mtp,
            tc.tile_pool(name="tuv", bufs=5) as tp,
            tc.tile_pool(name="zbp", bufs=6) as zbp,
            tc.tile_pool(name="stat", bufs=2) as sp,
            tc.tile_pool(name="psA", bufs=2, space="PSUM") as psA,
            tc.tile_pool(name="psB", bufs=2, space="PSUM") as psB,
            tc.tile_pool(name="psC", bufs=2, space="PSUM") as psC,
        ):
            # ---- resident constants
            adj_sb = cp.tile([128, N_BLK, 128], BF16, tag="adj")
            nc.gpsimd.dma_start(adj_sb[:], adj_d[:])
            w0_sb = cp.tile([AUG_C, HID], BF16, tag="w0")
            nc.gpsimd.dma_start(w0_sb[:], w0_d[:])
            wv_sb = cp.tile([AUG_C, 1], BF16, tag="wv")
            nc.gpsimd.dma_start(wv_sb[:], wv_d[:])
            wl_sb = []
            for k, d in enumerate(wl_d):
                w = cp.tile([HID, HID], BF16, tag=f"w{k + 1}")
                nc.gpsimd.dma_start(w[:], d[:])
                wl_sb.append(w)
            wo_sb = cp.tile([HID, OUT_C], BF16, tag="wo")
            nc.gpsimd.dma_start(wo_sb[:], wo_d[:])
            g_sb, be_sb = [], []
            for k in range(4):
                g = cp.tile([128, PAIR, HID], BF16, tag=f"g{k}")
                nc.gpsimd.dma_start(g[:], g_d[k][:])
                g_sb.append(g)
                b_ = cp.tile([128, PAIR, HID], BF16, tag=f"be{k}")
                nc.gpsimd.dma_start(b_[:], be_d[k][:])
                be_sb.append(b_)
            go_sb = cp.tile([128, 1], F32, tag="go")
            nc.gpsimd.dma_start(go_sb[:], go_d[:])
            beo_sb = cp.tile([128, 1], F32, tag="beo")
            nc.gpsimd.dma_start(beo_sb[:], beo_d[:])
            idb_sb = cp.tile([128, 128], BF16, tag="idb")
            nc.gpsimd.dma_start(idb_sb[:], idb_d[:])

            def rsqrt_chain(vh_ap, tag):
                """rstd = 1/sqrt(2*vh) via bit-trick seed + 1 Newton iter.
                vh_ap: [128, NT] f32 SBUF AP holding v/2 (+eps/2), > 0.
                Returns a [128, NT] f32 tile."""
                shape = [128, NT]
                sh = sp.tile(shape, I32, tag=f"{tag}_sh")
                nc.vector.tensor_scalar(
                    sh[:], vh_ap.bitcast(I32), 1, None, op0=ALU.arith_shift_right
                )
                nx = sp.tile(shape, I32, tag=f"{tag}_nx")
                nc.vector.tensor_scalar(nx[:], sh[:], -1, None, op0=ALU.bitwise_xor)
                y0b = sp.tile(shape, I32, tag=f"{tag}_y0")
                nc.vector.tensor_scalar(y0b[:], nx[:], MAGIC + 1, None, op0=ALU.add)
                y0 = y0b[:].bitcast(F32)
                t1 = sp.tile(shape, F32, tag=f"{tag}_t1")
                nc.vector.tensor_tensor(t1[:], y0, y0, op=ALU.mult)
                t2 = sp.tile(shape, F32, tag=f"{tag}_t2")
                nc.vector.tensor_tensor(t2[:], t1[:], vh_ap, op=ALU.mult)
                t3 = sp.tile(shape, F32, tag=f"{tag}_t3")
                nc.vector.tensor_scalar(
                    t3[:], t2[:], -1.0, 1.5, op0=ALU.mult, op1=ALU.add
                )
                rstd = sp.tile(shape, F32, tag=f"{tag}_rs")
                nc.vector.tensor_tensor(rstd[:], y0, t3[:], op=ALU.mult)
                return rstd

            def rstd_from_stats(st, m2_coef, tag):
                """st: [128, NGRP, GRP, 2, 3] bn_stats super-tile
                ((count,mean,M2) x even/odd). Combine (mean_total ~ 0):
                v = (M2e + M2o + (C/2)*(me^2+mo^2)) / C; vh = v/2 + eps/2.
                m2_coef = 1/(2C). Returns rstd [128, NT] f32 tile."""
                means = st[:, :, :, :, 1]
                m2s = st[:, :, :, :, 2]
                sq = sp.tile([128, NGRP, GRP, 2], F32, tag=f"{tag}_sq")
                nc.vector.tensor_tensor(sq[:], means, means, op=ALU.mult)
                r2 = sp.tile([128, NT], F32, tag=f"{tag}_r2")
                nc.vector.tensor_reduce(
                    r2[:], sq[:], axis=mybir.AxisListType.X, op=ALU.add
                )
                r1 = sp.tile([128, NT], F32, tag=f"{tag}_r1")
                nc.vector.tensor_reduce(
                    r1[:], m2s, axis=mybir.AxisListType.X, op=ALU.add
                )
                a = sp.tile([128, NT], F32, tag=f"{tag}_a")
                nc.vector.tensor_scalar(
                    a[:], r2[:], 0.25, LN_EPS * 0.5, op0=ALU.mult, op1=ALU.add
                )
                vh = sp.tile([128, NT], F32, tag=f"{tag}_vh")
                nc.vector.scalar_tensor_tensor(
                    vh[:], r1[:], m2_coef, a[:], op0=ALU.mult, op1=ALU.add
                )
                return rsqrt_chain(vh[:], tag)

            def gmul_early(zb2, gB, width):
                """qg = zb2 * gB on GPSIMD, issued early (overlaps stats)."""
                qg = tp.tile([128, PAIR, width], BF16, tag="qg")
                nc.gpsimd.tensor_tensor(qg[:], zb2[:], gB[:], op=ALU.mult)
                return qg

            def affine_gelu(qg, rstd, p, beB, out_ap, width):
                """out = gelu(qg * rstd[:, 8p:8p+8] bcast + beB). The be-add
                alternates DVE/GPSIMD to balance engine load."""
                rb = rstd[:, p * PAIR : (p + 1) * PAIR, None].broadcast_to(
                    [128, PAIR, width]
                )
                t = tp.tile([128, PAIR, width], BF16, tag="t")
                nc.vector.tensor_tensor(t[:], qg[:], rb, op=ALU.mult)
                v2 = tp.tile([128, PAIR, width], BF16, tag="v2")
                eng = nc.vector if p % 2 == 0 else nc.gpsimd
                eng.tensor_tensor(v2[:], t[:], beB[:], op=ALU.add)
                nc.scalar.activation(out_ap, v2[:], AF.Gelu)

            # pair p of a stage's affines enables these matmul groups of the
            # next stage (group g needs h tiles 4g-2 .. 4g+5)
            ENABLE = {0: [0], 1: [1, 2], 2: [3, 4], 3: [5, 6, 7]}

            def layer_mm_group(l, g, hsrc, st4, zbs):
                """strips + mt copy + W matmul + zb copy + bn_stats (+ early
                g-mult on the completed pair)."""
                mp = psA.tile([128, GRP, 128], F32, tag="mp")
                strips = STRIPS[g]
                for k, (i, j0, j1, off) in enumerate(strips):
                    nc.tensor.matmul(
                        mp[:, j0 - 4 * g : j1 - 4 * g + 1, :],
                        lhsT=hsrc[:, i, :],
                        rhs=adj_sb[:, off : off + (j1 - j0 + 1), :],
                        start=(k == 0), stop=(k == len(strips) - 1),
                    )
                mt = mtp.tile([128, GRP, 128], BF16, tag="mt")
                nc.scalar.activation(mt[:], mp[:], AF.Copy)
                zp = psB.tile([128, GRP, HID], F32, tag="zp")
                for jj in range(GRP):
                    nc.tensor.matmul(
                        zp[:, jj, :], lhsT=mt[:, jj, :],
                        rhs=wl_sb[l - 1][:], start=True, stop=True,
                    )
                if g % 2 == 0:
                    zb2n = zbp.tile([128, PAIR, HID], BF16, tag="zb")
                    zbs.append(zb2n)
                zb2 = zbs[-1]
                half = (g % 2) * GRP
                nc.scalar.activation(zb2[:, half : half + GRP, :], zp[:], AF.Copy)
                for jj in range(GRP):
                    nc.vector.bn_stats(st4[:, g, jj, :, :], zb2[:, half + jj, :])
                if g % 2 == 1:
                    zbs[-1] = gmul_early(zbs[-1], g_sb[l], HID)

            def head_mm_pair(p, hprev, h0, stH, qbs):
                s2 = mtp.tile([128, PAIR, 128], BF16, tag="s")
                nc.gpsimd.tensor_tensor(
                    s2[:], hprev[:, p * PAIR : (p + 1) * PAIR, :],
                    h0[:, p * PAIR : (p + 1) * PAIR, :], op=ALU.add,
                )
                qb2n = zbp.tile([128, PAIR, OUT_C], BF16, tag="qb")
                qbs.append(qb2n)
                for gg in range(2):
                    g = p * 2 + gg
                    stp = psC.tile([128, GRP, 128], BF16, tag="stp")
                    for jj in range(GRP):
                        nc.tensor.transpose(
                            stp[:, jj, :], s2[:, gg * GRP + jj, :], idb_sb[:]
                        )
                    sth = mtp.tile([128, GRP, 128], BF16, tag="mt")
                    nc.scalar.activation(sth[:], stp[:], AF.Copy)
                    qp = psB.tile([128, GRP, OUT_C], F32, tag="zp")
                    for jj in range(GRP):
                        nc.tensor.matmul(
                            qp[:, jj, :], lhsT=sth[:, jj, :], rhs=wo_sb[:],
                            start=True, stop=True,
                        )
                    qb2 = qbs[-1]
                    half = gg * GRP
                    nc.scalar.activation(
                        qb2[:, half : half + GRP, :], qp[:], AF.Copy
                    )
                    for jj in range(GRP):
                        nc.vector.bn_stats(stH[:, g, jj, :, :], qb2[:, half + jj, :])

            def head_affine(p, qb2, rstdH, out_sb):
                rb = rstdH[:, p * PAIR : (p + 1) * PAIR, None].broadcast_to(
                    [128, PAIR, OUT_C]
                )
                th = tp.tile([128, PAIR, OUT_C], BF16, tag="th")
                nc.gpsimd.tensor_tensor(th[:], qb2[:], rb, op=ALU.mult)
                qtp = psC.tile([128, GRP, 128], BF16, tag="stp")
                for k in range(GRP):
                    nc.tensor.transpose(
                        qtp[:, k, :], th[:, 2 * k : 2 * k + 2, :], idb_sb[:]
                    )
                nc.vector.tensor_scalar(
                    out_sb[:, p * GRP : (p + 1) * GRP, :], qtp[:],
                    go_sb[:], beo_sb[:], op0=ALU.mult, op1=ALU.add,
                )

            for b in range(B_LOC):
                xb = xp.tile([AUG_C, NT, 128], BF16, tag="xb")
                nc.gpsimd.dma_start(xb[:], x_d[b])
                h0 = hp.tile([128, NT, HID], BF16, tag="h0")
                ha = hp.tile([128, NT, HID], BF16, tag="ha")
                hb = hp.tile([128, NT, HID], BF16, tag="hb")
                out_sb = op_.tile([128, NT // 2, 128], F32, tag="out_sb")

                # ---- embed matmul phase: variance via PE (quadratic channels)
                vp = psC.tile([128, NT], F32, tag="vp")
                for j in range(NT):
                    nc.tensor.matmul(
                        vp[:, j : j + 1], lhsT=xb[:, j, :], rhs=wv_sb[:],
                        start=True, stop=True,
                    )
                vh0 = sp.tile([128, NT], F32, tag="vh0")
                nc.vector.tensor_scalar(
                    vh0[:], vp[:], 0.5, LN_EPS * 0.5, op0=ALU.mult, op1=ALU.max
                )
                rstd0 = rsqrt_chain(vh0[:], "c")
                ebs = []
                for g in range(NGRP):
                    ep = psB.tile([128, GRP, HID], F32, tag="zp")
                    for jj in range(GRP):
                        nc.tensor.matmul(
                            ep[:, jj, :], lhsT=xb[:, g * GRP + jj, :], rhs=w0_sb[:],
                            start=True, stop=True,
                        )
                    if g % 2 == 0:
                        eb2 = zbp.tile([128, PAIR, HID], BF16, tag="zb")
                        ebs.append(eb2)
                    half = (g % 2) * GRP
                    nc.scalar.activation(
                        ebs[-1][:, half : half + GRP, :], ep[:], AF.Copy
                    )
                    if g % 2 == 1:
                        ebs[-1] = gmul_early(ebs[-1], g_sb[0], HID)

                # ---- pipelined stages: affines of stage s interleave with
                # matmul groups of stage s+1
                hs = {0: h0, 1: ha, 2: hb, 3: ha}
                qgs, rstd = ebs, rstd0
                for l in (1, 2, 3):
                    hsrc, hdst = hs[l - 1], hs[l]
                    st4 = sp.tile([128, NGRP, GRP, 2, 3], F32, tag="st4")
                    zbs = []
                    for p in range(NPAIR):
                        affine_gelu(
                            qgs[p], rstd, p, be_sb[l - 1],
                            hsrc[:, p * PAIR : (p + 1) * PAIR, :], HID,
                        )
                        for g in ENABLE[p]:
                            layer_mm_group(l, g, hsrc, st4, zbs)
                    rstd = rstd_from_stats(st4, 1.0 / 256, "c")
                    qgs = zbs

                # l3 affines interleaved with head matmul pairs
                stH = sp.tile([128, NGRP, GRP, 2, 3], F32, tag="stH")
                qbs = []
                for p in range(NPAIR):
                    affine_gelu(
                        qgs[p], rstd, p, be_sb[3],
                        hs[3][:, p * PAIR : (p + 1) * PAIR, :], HID,
                    )
                    head_mm_pair(p, hs[3], h0, stH, qbs)
                rstdH = rstd_from_stats(stH, 1.0 / 128, "c")
                for p in range(NPAIR):
                    head_affine(p, qbs[p], rstdH, out_sb)
                nc.gpsimd.dma_start(out_d[b][:, :, 0, :], out_sb[0:OUT_C, :, :])
                nc.gpsimd.dma_start(out_d[b][:, :, 1, :], out_sb[OUT_C:128, :, :])

    n = _split_multi_waits(nc)
    print(f"kernel: split {n} multi-wait instructions")
    return nc


_NC_CACHE = None


def _get_nc():
    global _NC_CACHE
    if _NC_CACHE is None:
        _NC_CACHE = _build_program()
    return _NC_CACHE


# -------------------------------------------------------------- host wrapper
def _prep_inputs(x, adj, W0, W1, W2, W3, Wo, gs, bes, go, beo):
    bf = ml_dtypes.bfloat16
    # adjacency strip blocks -> [128, N_BLK, 128]
    blocks = np.empty((N_BLK, 128, 128), np.float32)
    for (i, j), s in ADJ_SLOTS.items():
        blocks[s] = adj[128 * i : 128 * (i + 1), 128 * j : 128 * (j + 1)]
    adjS = np.ascontiguousarray(blocks.transpose(1, 0, 2)).astype(bf)

    P128 = np.eye(HID, dtype=np.float32) - 1.0 / HID
    P64 = np.eye(OUT_C, dtype=np.float32) - 1.0 / OUT_C

    W0c = W0.astype(np.float32) @ P128                       # [3, 128]
    w0aug = np.zeros((AUG_C, HID), np.float32)
    w0aug[:IN_C] = W0c
    Q0 = (W0c @ W0c.T) / HID                                 # [3, 3]
    wv = np.array(
        [0, 0, 0, Q0[0, 0], Q0[1, 1], Q0[2, 2],
         2 * Q0[0, 1], 2 * Q0[0, 2], 2 * Q0[1, 2]], np.float32
    ).reshape(AUG_C, 1)

    def rep(v, width):
        return np.ascontiguousarray(
            np.broadcast_to(v.astype(np.float32), (128, PAIR, width))
        ).astype(bf)

    common = {
        "adjS": adjS,
        "w0aug": w0aug.astype(bf),
        "wv": wv.astype(bf),
        "w1": (W1.astype(np.float32) @ P128).astype(bf),
        "w2": (W2.astype(np.float32) @ P128).astype(bf),
        "w3": (W3.astype(np.float32) @ P128).astype(bf),
        "wo": (Wo.astype(np.float32) @ P64).astype(bf),
        "go": np.tile(go.astype(np.float32).reshape(OUT_C, 1), (2, 1)),
        "beo": np.tile(beo.astype(np.float32).reshape(OUT_C, 1), (2, 1)),
        "id_bf": np.eye(128, dtype=np.float32).astype(bf),
    }
    for k in range(4):
        common[f"g{k}B"] = rep(gs[k], HID)
        common[f"be{k}B"] = rep(bes[k], HID)

    # augmented input channels: [x, x^2 monomials]
    xn = x.reshape(B, IN_C, N).astype(np.float32)
    xaug = np.empty((B, AUG_C, N), np.float32)
    xaug[:, :IN_C] = xn
    xaug[:, 3] = xn[:, 0] * xn[:, 0]
    xaug[:, 4] = xn[:, 1] * xn[:, 1]
    xaug[:, 5] = xn[:, 2] * xn[:, 2]
    xaug[:, 6] = xn[:, 0] * xn[:, 1]
    xaug[:, 7] = xn[:, 0] * xn[:, 2]
    xaug[:, 8] = xn[:, 1] * xn[:, 2]
    xr = xaug.reshape(B, AUG_C, NT, 128).astype(bf)

    in_maps = []
    for c in range(N_CORES):
        m = dict(common)
        m["xaug"] = np.ascontiguousarray(xr[c * B_LOC : (c + 1) * B_LOC])
        in_maps.append(m)
    return in_maps


def kernel(x, adj, W0, b0, g0, be0, W1, g1, be1, W2, g2, be2, W3, g3, be3,
           Wo, bo, go, beo, _trace=False):
    x = np.asarray(x, np.float32)
    adj = np.asarray(adj, np.float32)
    in_maps = _prep_inputs(
        x, adj,
        np.asarray(W0), np.asarray(W1), np.asarray(W2), np.asarray(W3),
        np.asarray(Wo),
        [np.asarray(g0), np.asarray(g1), np.asarray(g2), np.asarray(g3)],
        [np.asarray(be0), np.asarray(be1), np.asarray(be2), np.asarray(be3)],
        np.asarray(go), np.asarray(beo),
    )
    nc = _get_nc()
    res = bass_utils.run_bass_kernel_spmd(
        nc, in_maps, core_ids=list(range(N_CORES)), trace=_trace
    )
    out = np.concatenate(
        [res.results[c]["out"].reshape(B_LOC, OUT_C, GRID, GRID)
         for c in range(N_CORES)], axis=0
    )
    if _trace:
        kernel._last_result = res
    return out
